# revision 1
# baseline (speedup 1.0000x reference)
"""Multi-head attention (B=2, S=2048, D=1024, H=16) on 8 Trainium2 cores.

Sharding: (batch, head-group-of-4) -> 8 cores, Megatron-style. Core c
handles batch c//4 and heads 4*(c%4)..4*(c%4)+3 (d_local = 256 columns of
Wq/Wk/Wv, 256 rows of Wo). Each core computes a partial [2048, 1024]
output; the host sums the 4 partials per batch (row-parallel Wo).

Key-side truncation: softmax keys are masked per batch to valid_lens;
only ceil(max(valid_lens)/128) key tiles are ever computed (the rest
contribute exp(-1e6) = 0). The mask is applied as a per-partition bias
on the ScalarE exp that evacuates score PSUM tiles (scores are computed
transposed: [key, query]).

Precision: activations/weights stream in as fp16 (inputs are ~N(0,1), so
fp16's 11-bit mantissa costs ~5e-5 rel per element); all matmuls run
single-pass (fp16 or raw-fp32 "float32r", 1 cycle/row); PSUM accumulates
fp32. Partial outputs return as fp16 and are summed in fp32 on host.

The kernel program is built at call time from the actual valid_lens, so
any input values work; shapes are hardcoded to this problem.
"""
import sys
if "/opt/trn_rl_repo" not in sys.path:
    sys.path.insert(0, "/opt/trn_rl_repo")
import os
import time
import numpy as np

B, SQ, SK, D, H, HD = 2, 2048, 2048, 1024, 16, 64
NEG = -1.0e6
N_CORES = 8
DL = 256          # d_local: 4 heads * 64
KD = D // 128     # contraction tiles over D

_NC_CACHE = {}
last_results = None
last_exec_wall_s = None

# "f16": fp16 streams and fp16 attention core (fast path; FWL weight loads)
# "f32r": all-fp32 storage, single-pass raw-fp32 matmuls
# "f32": exact fp32 (4 cycles/row matmuls)
PREC = os.environ.get("BASS_MHA_PREC", "f16")


def _build(KT, prec=None):
    import concourse.bass as bass  # noqa: F401
    import concourse.tile as tile
    from concourse import bacc, mybir

    prec = PREC if prec is None else prec
    f32 = mybir.dt.float32
    f16 = mybir.dt.float16
    # matmul-operand dtype for the attention core (scores/ctx/out-proj).
    # fp16 (not float32r) so LDWEIGHTS gets the fast-weight-load path on HW.
    md = {"f32": f32, "f32r": mybir.dt.float32r, "f16": f16}[prec]
    # dtype of the streamed activations/weights (and their matmuls)
    xd = f16 if prec == "f16" else md
    # output dtype
    od = f16 if prec == "f16" else f32

    LK = KT * 128
    kchunks = [(i * 512, min(512, LK - i * 512)) for i in range((LK + 511) // 512)]

    nc = bacc.Bacc("TRN2", target_bir_lowering=False, debug=False,
                   num_devices=N_CORES)
    xqT = nc.dram_tensor("xqT", [D, SQ], xd, kind="ExternalInput")
    xkT = nc.dram_tensor("xkT", [D, LK], xd, kind="ExternalInput")
    xvT = nc.dram_tensor("xvT", [D, LK], xd, kind="ExternalInput")
    wq = nc.dram_tensor("wq", [D, DL], xd, kind="ExternalInput")
    wk = nc.dram_tensor("wk", [D, DL], xd, kind="ExternalInput")
    wv = nc.dram_tensor("wv", [D, DL], xd, kind="ExternalInput")
    wo = nc.dram_tensor("wo", [DL, D], md, kind="ExternalInput")
    mask = nc.dram_tensor("mask", [128, KT], f32, kind="ExternalInput")
    out = nc.dram_tensor("out", [SQ, D], od, kind="ExternalOutput")
    dbg = os.environ.get("BASS_MHA_DEBUG") == "1"
    if dbg:
        dbg_qt = nc.dram_tensor("dbg_qt", [128, 2, SQ], md, kind="ExternalOutput")
        dbg_kt = nc.dram_tensor("dbg_kt", [128, 2, LK], md, kind="ExternalOutput")
        dbg_v = nc.dram_tensor("dbg_v", [128, KT, 4, 128], md, kind="ExternalOutput")

    with tile.TileContext(nc) as tc:
        with tc.tile_pool(name="singles", bufs=1) as sg:
            wq_sb = sg.tile([128, KD, DL], xd)
            wk_sb = sg.tile([128, KD, DL], xd)
            wv_sb = sg.tile([128, KD, DL], xd)
            wo_sb = sg.tile([128, DL // 128, D], md)
            mask_sb = sg.tile([128, KT], f32)
            kt_sb = sg.tile([128, 2, LK], md)       # K^T  [d_local, key]
            v_sb = sg.tile([128, KT, 4, 128], md)   # V''  [key, head, d | ones]
            qt_sb = sg.tile([128, 2, SQ], md)       # Q^T  [d_local, query]
            ctxT_sb = sg.tile([128, 2, SQ], md)     # Ctx^T normalized

            # DMA issue order = arrival order: K path, Q path, V path, Wo
            nc.sync.dma_start(out=mask_sb, in_=mask[:, :])
            nc.sync.dma_start(out=wk_sb, in_=wk[:, :].rearrange("(k p) j -> p k j", p=128))
            if md == f16:
                nc.vector.memset(v_sb, 1.0)
            else:
                nc.vector.memset(v_sb.bitcast(f32), 1.0)
            # dummy exp: pulls the ~2.7us activation-table load off phase C's
            # critical path (ACT is otherwise idle until the first softmax)
            warm_sb = sg.tile([1, 1], f32)
            nc.scalar.activation(warm_sb, mask_sb[0:1, 0:1],
                                 mybir.ActivationFunctionType.Exp)

            # ---- resident input streams (DMA priority: xk, xq, xv) ----
            strm_cm = tc.tile_pool(name="streams", bufs=1)
            strm = strm_cm.__enter__()
            xk_full = strm.tile([128, KD, LK], xd)
            xq_full = strm.tile([128, KD, SQ], xd)
            xv_full = strm.tile([128, KD, LK], xd)
            for k in range(KD):
                nc.sync.dma_start(out=xk_full[:, k, :],
                                  in_=xkT[k * 128:(k + 1) * 128, :])

            # ---- Phase A1: K^T = (Wk^T blocks) @ Xk^T, [256, LK] ----
            with tc.tile_pool(name="psA", bufs=1, space="PSUM") as psA:
                accs = {}
                for m in range(2):
                    for ci, (c0, cw) in enumerate(kchunks):
                        accs[(m, ci)] = psA.tile([128, cw], f32,
                                                 tag=f"kt{m}_{ci}", name=f"kt{m}_{ci}")
                for k in range(KD):
                    for m in range(2):
                        for ci, (c0, cw) in enumerate(kchunks):
                            nc.tensor.matmul(accs[(m, ci)],
                                             wk_sb[:, k, m * 128:(m + 1) * 128],
                                             xk_full[:, k, c0:c0 + cw],
                                             start=(k == 0), stop=(k == KD - 1))
                for m in range(2):
                    for ci, (c0, cw) in enumerate(kchunks):
                        nc.vector.tensor_copy(kt_sb[:, m, c0:c0 + cw], accs[(m, ci)])

            # ---- Phases B + A2: Q^T and V projections riding the DMA ----
            # xq streams in right after xk; Q accumulates in narrow passes
            # (2 or 4 PSUM banks) so the V accumulators (1 bank per key
            # tile, bank-aligned: matmul start=True clears a whole bank)
            # fit alongside. Pass 0 rides the xq stream; V rides xv.
            nc.sync.dma_start(out=wq_sb, in_=wq[:, :].rearrange("(k p) j -> p k j", p=128))
            if KT <= 8:
                qpass, nchunk = 2, 2
            else:
                qpass, nchunk = 4, 1
            maxg = 8 - 2 * nchunk
            vgroups = [list(range(g0, min(g0 + maxg, KT)))
                       for g0 in range(0, KT, maxg)]
            with tc.tile_pool(name="psB", bufs=1, space="PSUM") as psB, \
                 tc.tile_pool(name="psV", bufs=1, space="PSUM") as psV:
                for k in range(KD):
                    nc.sync.dma_start(out=xq_full[:, k, :],
                                      in_=xqT[k * 128:(k + 1) * 128, :])
                nc.sync.dma_start(out=wv_sb, in_=wv[:, :].rearrange("(k p) j -> p k j", p=128))
                for k in range(KD):
                    nc.sync.dma_start(out=xv_full[:, k, :],
                                      in_=xvT[k * 128:(k + 1) * 128, :])
                for p in range(qpass):
                    chunks = list(range(p * nchunk, (p + 1) * nchunk))
                    qaccs = {}
                    for m in range(2):
                        for c in chunks:
                            qaccs[(m, c)] = psB.tile([128, 512], f32,
                                                     tag=f"q{m}_{c % nchunk}",
                                                     name=f"qp{p}_{m}_{c}")
                    for k in range(KD):
                        for m in range(2):
                            for c in chunks:
                                nc.tensor.matmul(
                                    qaccs[(m, c)],
                                    wq_sb[:, k, m * 128:(m + 1) * 128],
                                    xq_full[:, k, c * 512:(c + 1) * 512],
                                    start=(k == 0), stop=(k == KD - 1))
                    if p < len(vgroups):
                        g = vgroups[p]
                        vacc = psV.tile([128, len(g), 512], f32, tag="vg",
                                        name=f"vg{p}")
                        for k in range(KD):
                            for vi, t in enumerate(g):
                                nc.tensor.matmul(
                                    vacc[:, vi, 0:DL],
                                    xv_full[:, k, t * 128:(t + 1) * 128],
                                    wv_sb[:, k, :],
                                    start=(k == 0), stop=(k == KD - 1),
                                    skip_group_check=True)
                        for vi, t in enumerate(g):
                            for hh in range(4):
                                nc.vector.tensor_copy(
                                    v_sb[:, t, hh, 0:64],
                                    vacc[:, vi, hh * 64:(hh + 1) * 64])
                    for m in range(2):
                        for c in chunks:
                            nc.vector.tensor_copy(
                                qt_sb[:, m, c * 512:(c + 1) * 512], qaccs[(m, c)])

            nc.sync.dma_start(out=wo_sb, in_=wo[:, :].rearrange("(k p) j -> p k j", p=128))

            # ---- Phase C: per-head attention ----
            # scores^T tile = K^T_h.T @ Q^T_h  -> exp(bias=mask) -> P^T
            # ctx'' = V''.T @ P^T : rows 0-63 ctx, rows 64-127 denominator
            with tc.tile_pool(name="pt", bufs=8) as ptp, \
                 tc.tile_pool(name="misc", bufs=4) as mp, \
                 tc.tile_pool(name="psC", bufs=2, space="PSUM") as psC, \
                 tc.tile_pool(name="psS", bufs=2, space="PSUM") as psS:
                for half in range(2):
                    for hh in range(4):
                        mt, mo = hh // 2, 64 * (hh % 2)
                        h0 = half * 1024
                        ctx_ps = psC.tile([128, 1024], f32, tag="ctx")
                        for t in range(KT):
                            pt_t = ptp.tile([128, 1024], md, tag="pt")
                            s_ps = psS.tile([128, 1024], f32, tag="s")
                            for cq in range(2):
                                nc.tensor.matmul(
                                    s_ps[:, cq * 512:(cq + 1) * 512],
                                    kt_sb[mo:mo + 64, mt, t * 128:(t + 1) * 128],
                                    qt_sb[mo:mo + 64, mt, h0 + cq * 512:h0 + (cq + 1) * 512],
                                    start=True, stop=True)
                            nc.scalar.activation(
                                pt_t, s_ps,
                                mybir.ActivationFunctionType.Exp,
                                bias=mask_sb[:, t:t + 1], scale=0.125)
                            for c in range(2):
                                nc.tensor.matmul(ctx_ps[:, c * 512:(c + 1) * 512],
                                                 v_sb[:, t, hh, :],
                                                 pt_t[:, c * 512:(c + 1) * 512],
                                                 start=(t == 0), stop=(t == KT - 1),
                                                 skip_group_check=True)
                        # rows 64-127 of ctx_ps all hold the softmax denominator
                        rcb = mp.tile([64, 1024], f32, tag="rcb")
                        nc.vector.reciprocal(rcb, ctx_ps[64:128, :])
                        nc.vector.tensor_mul(ctxT_sb[mo:mo + 64, mt, h0:h0 + 1024],
                                             ctx_ps[0:64, :], rcb)

            # ---- Phase D: partial output projection ----
            with tc.tile_pool(name="po", bufs=6) as pop, \
                 tc.tile_pool(name="psD", bufs=4, space="PSUM") as psD:
                for qi in range(SQ // 128):
                    o_ps = psD.tile([128, D], f32, tag="o")
                    for n in range(2):
                        for kk in range(2):
                            nc.tensor.matmul(o_ps[:, n * 512:(n + 1) * 512],
                                             ctxT_sb[:, kk, qi * 128:(qi + 1) * 128],
                                             wo_sb[:, kk, n * 512:(n + 1) * 512],
                                             start=(kk == 0), stop=(kk == 1))
                    o_sb = pop.tile([128, D], od, tag="o_sb")
                    nc.scalar.copy(o_sb[:, 0:512], o_ps[:, 0:512])
                    nc.vector.tensor_copy(o_sb[:, 512:1024], o_ps[:, 512:1024])
                    nc.sync.dma_start(out=out[qi * 128:(qi + 1) * 128, :], in_=o_sb)
            strm_cm.__exit__(None, None, None)
            if dbg:
                nc.sync.dma_start(out=dbg_qt[:, :, :], in_=qt_sb)
                nc.sync.dma_start(out=dbg_kt[:, :, :], in_=kt_sb)
                nc.sync.dma_start(out=dbg_v[:, :, :, :], in_=v_sb)
    nc.compile()
    return nc


def kernel(**inputs):
    global last_results, last_exec_wall_s
    from concourse.bass_utils import run_bass_kernel_spmd

    # BASS_TRACE needs the axon NTFF hook; disable tracing when the hook
    # module is unavailable so a stray env var cannot crash the run.
    if os.environ.get("BASS_TRACE"):
        try:
            from antenv import axon_hooks  # noqa: F401
        except Exception:
            os.environ["BASS_NEVER_TRACE"] = "1"

    q = np.asarray(inputs["queries"], dtype=np.float32)
    kx = np.asarray(inputs["keys"], dtype=np.float32)
    vx = np.asarray(inputs["values"], dtype=np.float32)
    vl = np.asarray(inputs["valid_lens"], dtype=np.int64).reshape(B)
    Wq = np.asarray(inputs["Wq"], dtype=np.float32)
    Wk = np.asarray(inputs["Wk"], dtype=np.float32)
    Wv = np.asarray(inputs["Wv"], dtype=np.float32)
    Wo = np.asarray(inputs["Wo"], dtype=np.float32)
    assert q.shape == (B, SQ, D) and kx.shape == (B, SK, D) and vx.shape == (B, SK, D)

    lens = np.clip(vl, 1, SK)
    lmax = int(lens.max())
    KT = (lmax + 127) // 128
    LK = KT * 128

    key = (KT, PREC)
    if key not in _NC_CACHE:
        _NC_CACHE[key] = _build(KT)
    nc = _NC_CACHE[key]

    xdt = np.float16 if PREC == "f16" else np.float32

    in_maps = []
    for c in range(N_CORES):
        b, hg = c // 4, c % 4
        cols = slice(DL * hg, DL * (hg + 1))
        m = np.where(np.arange(LK) < lens[b], 0.0, NEG).astype(np.float32)
        in_maps.append({
            "xqT": np.ascontiguousarray(q[b].T.astype(xdt)),
            "xkT": np.ascontiguousarray(kx[b, :LK].T.astype(xdt)),
            "xvT": np.ascontiguousarray(vx[b, :LK].T.astype(xdt)),
            "wq": np.ascontiguousarray(Wq[:, cols].astype(xdt)),
            "wk": np.ascontiguousarray(Wk[:, cols].astype(xdt)),
            "wv": np.ascontiguousarray(Wv[:, cols].astype(xdt)),
            "wo": np.ascontiguousarray(Wo[cols, :].astype(xdt)),
            "mask": np.ascontiguousarray(m.reshape(KT, 128).T),
        })

    t0 = time.perf_counter()
    res = run_bass_kernel_spmd(nc, in_maps, core_ids=list(range(N_CORES)))
    last_exec_wall_s = time.perf_counter() - t0
    last_results = res

    outs = [res.results[c]["out"].astype(np.float32) for c in range(N_CORES)]
    full = np.stack([outs[0] + outs[1] + outs[2] + outs[3],
                     outs[4] + outs[5] + outs[6] + outs[7]])
    return full.astype(np.float32)



# revision 6
# speedup vs baseline: 1.0733x; 1.0733x over previous
"""Multi-head attention (B=2, S=2048, D=1024, H=16) on 8 Trainium2 cores.

Sharding: (batch, head-group-of-4) -> 8 cores, Megatron-style. Core c
handles batch c//4 and heads 4*(c%4)..4*(c%4)+3 (d_local = 256 columns of
Wq/Wk/Wv, 256 rows of Wo). Each core computes a partial [2048, 1024]
output; the host sums the 4 partials per batch (row-parallel Wo).

Key-side truncation: only ceil(max(valid_lens)/128) key tiles are ever
computed; invalid keys get a -1e6 bias on the ScalarE exp (scores are
computed transposed [key, query], denominator rides the ctx matmul as
64 "ones" columns of V'').

This build is organized as one deep pipeline so the ACT-engine exp
stream (the phase-C bottleneck) and the DMA streams hide under the
TensorE matmul stream (the global floor):
  warmup dummy matmuls (p-state ramp) -> K-proj (rides xk DMA) ->
  V-proj (rides per-key-tile xv DMA) -> Q-proj half 0 (rides xq) ->
  attention half 0 (with Q-proj half 1 matmuls injected into the
  per-key-tile slack) -> attention half 1 (with out-proj half 0
  injected) -> out-proj half 1, evacuations spread across ACT/DVE/Pool.

Precision: fp16 streams, fp16 single-pass matmuls, fp32 PSUM; partial
outputs return fp16 and are summed in fp32 on host.
"""
import sys
if "/opt/trn_rl_repo" not in sys.path:
    sys.path.insert(0, "/opt/trn_rl_repo")
import os
import time
import numpy as np

B, SQ, SK, D, H, HD = 2, 2048, 2048, 1024, 16, 64
NEG = -1.0e6
N_CORES = 8
DL = 256          # d_local: 4 heads * 64
KD = D // 128     # contraction tiles over D
N_WARM = int(os.environ.get("BASS_MHA_WARM", "4"))

_NC_CACHE = {}
last_results = None
last_exec_wall_s = None


def _chunks(total, cw):
    out = []
    c0 = 0
    while c0 < total:
        w = min(cw, total - c0)
        out.append((c0, w))
        c0 += w
    return out


def _build(KT):
    import concourse.bass as bass  # noqa: F401
    import concourse.tile as tile
    from concourse import bacc, mybir

    f32 = mybir.dt.float32
    f16 = mybir.dt.float16
    Exp = mybir.ActivationFunctionType.Exp
    LK = KT * 128

    nc = bacc.Bacc("TRN2", target_bir_lowering=False, debug=False,
                   num_devices=N_CORES)
    xqT = nc.dram_tensor("xqT", [D, SQ], f16, kind="ExternalInput")
    xkT = nc.dram_tensor("xkT", [D, LK], f16, kind="ExternalInput")
    xvT = nc.dram_tensor("xvT", [D, LK], f16, kind="ExternalInput")
    wq = nc.dram_tensor("wq", [D, DL], f16, kind="ExternalInput")
    wk = nc.dram_tensor("wk", [D, DL], f16, kind="ExternalInput")
    wv = nc.dram_tensor("wv", [D, DL], f16, kind="ExternalInput")
    wo = nc.dram_tensor("wo", [DL, D], f16, kind="ExternalInput")
    mask = nc.dram_tensor("mask", [128, KT], f32, kind="ExternalInput")
    out = nc.dram_tensor("out", [SQ, D], f16, kind="ExternalOutput")

    with tile.TileContext(nc) as tc:
        with tc.tile_pool(name="sg", bufs=1) as sg:
            wk_sb = sg.tile([128, KD, DL], f16)
            wq_sb = sg.tile([128, KD, DL], f16)
            wv_sb = sg.tile([128, KD, DL], f16)
            wo_sb = sg.tile([128, DL // 128, D], f16)
            mask_sb = sg.tile([128, KT], f32)
            kt_sb = sg.tile([128, 2, LK], f16)
            qt_sb = sg.tile([128, 2, SQ], f16)
            # V'' per head: [key, (v-tile | ones)] pairs per key tile
            v3_sb = sg.tile([128, 4, KT, 2, HD], f16)
            ctxT_sb = sg.tile([128, 2, SQ], f16)
            zero_sb = sg.tile([128, 512], f16)
            xk_sb = sg.tile([128, KD, LK], f16)
            xv_sb = sg.tile([128, KD, KT, 128], f16)
            xq_sb = sg.tile([128, KD, SQ], f16)
            warm_sb = sg.tile([1, 1], f32)

            # ---- DMA queue: arrival order is the pipeline order ----
            nc.sync.dma_start(out=mask_sb, in_=mask[:, :])
            for c in range(2):
                nc.sync.dma_start(
                    out=wk_sb[:, c * 4:(c + 1) * 4, :],
                    in_=wk[c * 512:(c + 1) * 512, :].rearrange(
                        "(k p) j -> p k j", p=128))
            for c in range(4):
                nc.sync.dma_start(
                    out=xk_sb[:, 2 * c:2 * c + 2, :],
                    in_=xkT[c * 256:(c + 1) * 256, :].rearrange(
                        "(k p) j -> p k j", p=128))
            nc.sync.dma_start(out=wv_sb,
                              in_=wv[:, :].rearrange("(k p) j -> p k j", p=128))
            for t in range(KT):
                nc.sync.dma_start(
                    out=xv_sb[:, :, t, :],
                    in_=xvT[:, t * 128:(t + 1) * 128].rearrange(
                        "(k p) j -> p k j", p=128))
            nc.sync.dma_start(out=wq_sb,
                              in_=wq[:, :].rearrange("(k p) j -> p k j", p=128))
            for half in range(2):
                for c in range(4):
                    nc.sync.dma_start(
                        out=xq_sb[:, 2 * c:2 * c + 2,
                                  half * 1024:(half + 1) * 1024],
                        in_=xqT[c * 256:(c + 1) * 256,
                                half * 1024:(half + 1) * 1024].rearrange(
                                    "(k p) j -> p k j", p=128))
            nc.sync.dma_start(out=wo_sb,
                              in_=wo[:, :].rearrange("(k p) j -> p k j", p=128))

            nc.vector.memset(zero_sb, 0.0)
            nc.vector.memset(v3_sb, 1.0)  # ones slots; v slots overwritten
            # preload the exp activation table while DMAs stream
            nc.scalar.activation(warm_sb, mask_sb[0:1, 0:1], Exp)

            # ---- warmup: ramp the PE p-state during the DMA lead-in ----
            if N_WARM:
                wp_cm = tc.tile_pool(name="wp", bufs=1, space="PSUM")
                wp = wp_cm.__enter__()
                wps = wp.tile([128, 512], f32, tag="w")
                for _ in range(N_WARM):
                    nc.tensor.matmul(wps, zero_sb[:, 0:128], zero_sb,
                                     start=True, stop=True,
                                     skip_group_check=True)
                wp_cm.__exit__(None, None, None)

            # ---- A: K^T = (Wk^T blocks) @ Xk^T, k-major to ride xk DMA ----
            psA_cm = tc.tile_pool(name="psA", bufs=1, space="PSUM")
            psA = psA_cm.__enter__()
            for sec0, secw in _chunks(LK, 1024):
                am = [psA.tile([128, 1024], f32, tag="a", bufs=2,
                               name=f"a{m}_{sec0}") for m in range(2)]
                for k in range(KD):
                    for m in range(2):
                        for c0, cw in _chunks(secw, 512):
                            nc.tensor.matmul(
                                am[m][:, c0:c0 + cw],
                                wk_sb[:, k, m * 128:(m + 1) * 128],
                                xk_sb[:, k, sec0 + c0:sec0 + c0 + cw],
                                start=(k == 0), stop=(k == KD - 1))
                for m in range(2):
                    for c0, cw in _chunks(secw, 512):
                        nc.vector.tensor_copy(
                            kt_sb[:, m, sec0 + c0:sec0 + c0 + cw],
                            am[m][:, c0:c0 + cw])

            # ---- V-proj: per key tile, rides the per-tile xv DMA ----
            for tp in range(0, KT, 2):
                nj = min(2, KT - tp)
                vt = psA.tile([128, 2, 4, HD], f32, tag="a", bufs=2,
                              name=f"v{tp}")
                for j in range(nj):
                    t = tp + j
                    for k in range(KD):
                        nc.tensor.matmul(vt[:, j], xv_sb[:, k, t, :],
                                         wv_sb[:, k, :],
                                         start=(k == 0), stop=(k == KD - 1),
                                         skip_group_check=True)
                for j in range(nj):
                    t = tp + j
                    nc.vector.tensor_copy(v3_sb[:, :, t, 0, :], vt[:, j])
            psA_cm.__exit__(None, None, None)

            # ---- B half 0: Q^T cols 0:1024, k-major to ride xq DMA ----
            psS_cm = tc.tile_pool(name="psS", bufs=1, space="PSUM")
            psS = psS_cm.__enter__()
            qp = [psS.tile([128, 1024], f32, tag="s", bufs=2, name=f"q{m}")
                  for m in range(2)]
            for k in range(KD):
                for m in range(2):
                    for cq in range(2):
                        nc.tensor.matmul(
                            qp[m][:, cq * 512:(cq + 1) * 512],
                            wq_sb[:, k, m * 128:(m + 1) * 128],
                            xq_sb[:, k, cq * 512:(cq + 1) * 512],
                            start=(k == 0), stop=(k == KD - 1))
            for m in range(2):
                nc.scalar.copy(qt_sb[:, m, 0:1024], qp[m])

            psX_cm = tc.tile_pool(name="psX", bufs=1, space="PSUM")
            psX = psX_cm.__enter__()
            psC_cm = tc.tile_pool(name="psC", bufs=1, space="PSUM")
            psC = psC_cm.__enter__()
            ptp_cm = tc.tile_pool(name="ptp", bufs=1)
            ptp = ptp_cm.__enter__()
            otp_cm = tc.tile_pool(name="otp", bufs=1)
            otp = otp_cm.__enter__()
            rcp_cm = tc.tile_pool(name="rcp", bufs=1)
            rcp = rcp_cm.__enter__()

            # Q-proj matmuls for half 1 (cols 1024:2048), injected one at a
            # time into attention-half-0 slack; rides the late xq DMA.
            def bh1_quarter(qq):
                xp = psX.tile([128, 2, 512], f32, tag="x", name=f"b1_{qq}")
                thunks = []
                for k in range(KD):
                    for m in range(2):
                        def mm(k=k, m=m):
                            nc.tensor.matmul(
                                xp[:, m, :],
                                wq_sb[:, k, m * 128:(m + 1) * 128],
                                xq_sb[:, k, qq * 512:(qq + 1) * 512],
                                start=(k == 0), stop=(k == KD - 1),
                                skip_group_check=True)
                        thunks.append(mm)

                def evac(qq=qq, xp=xp):
                    nc.vector.tensor_copy(qt_sb[:, :, qq * 512:(qq + 1) * 512],
                                          xp)
                thunks.append(evac)
                return thunks

            # out-proj for one query tile; evac split DVE/Pool (half 0,
            # injected under attention half 1) or ACT/DVE (half 1 tail).
            def d_tile(qi, in_s_pool, tail):
                if in_s_pool:
                    ov = psS.tile([128, 2, 512], f32, tag="s", bufs=2,
                                  name=f"o{qi}")
                else:
                    ov = psX.tile([128, 2, 512], f32, tag="x", name=f"o{qi}")
                thunks = []
                for n in range(2):
                    for kk in range(2):
                        def mm(n=n, kk=kk):
                            nc.tensor.matmul(
                                ov[:, n, :],
                                ctxT_sb[:, kk, qi * 128:(qi + 1) * 128],
                                wo_sb[:, kk, n * 512:(n + 1) * 512],
                                start=(kk == 0), stop=(kk == 1),
                                skip_group_check=True)
                        thunks.append(mm)

                def evac():
                    ot = otp.tile([128, 1024], f16, tag="ot", bufs=4,
                                  name=f"ot{qi}")
                    if tail:
                        nc.scalar.copy(ot[:, 0:512], ov[:, 0, :])
                        nc.vector.tensor_copy(ot[:, 512:1024], ov[:, 1, :])
                    else:
                        nc.vector.tensor_copy(ot[:, 0:512], ov[:, 0, :])
                        nc.vector.tensor_copy(ot[:, 512:1024], ov[:, 1, :])
                    nc.sync.dma_start(out=out[qi * 128:(qi + 1) * 128, :],
                                      in_=ot)
                thunks.append(evac)
                return thunks

            # ---- attention block for (half, head): scores -> exp -> ctx ----
            def c_block(half, hh, inject):
                mt, mo = hh // 2, 64 * (hh % 2)
                h0 = half * 1024
                ctx = [psC.tile([128, 512], f32, tag="c", bufs=2,
                                name=f"c{half}_{hh}_{cq}") for cq in range(2)]
                sts, pts = [], []

                def s_step(t):
                    st = psS.tile([128, 1024], f32, tag="s", bufs=2,
                                  name=f"s{half}_{hh}_{t}")
                    for cq in range(2):
                        nc.tensor.matmul(
                            st[:, cq * 512:(cq + 1) * 512],
                            kt_sb[mo:mo + 64, mt, t * 128:(t + 1) * 128],
                            qt_sb[mo:mo + 64, mt,
                                  h0 + cq * 512:h0 + (cq + 1) * 512],
                            start=True, stop=True)
                    sts.append(st)

                def e_step(t):
                    pt = ptp.tile([128, 1024], f16, tag="pt", bufs=6,
                                  name=f"p{half}_{hh}_{t}")
                    nc.scalar.activation(pt, sts[t], Exp,
                                         bias=mask_sb[:, t:t + 1], scale=0.125)
                    pts.append(pt)

                def c_step(t):
                    for cq in range(2):
                        nc.tensor.matmul(
                            ctx[cq], v3_sb[:, hh, t, :, :],
                            pts[t][:, cq * 512:(cq + 1) * 512],
                            start=(t == 0), stop=(t == KT - 1),
                            skip_group_check=True)

                def drip(n):
                    for _ in range(n):
                        if inject:
                            inject.pop(0)()

                s_step(0)
                for t in range(1, KT):
                    s_step(t)
                    e_step(t - 1)
                    c_step(t - 1)
                    drip(2)
                e_step(KT - 1)
                c_step(KT - 1)
                drip(2)
                for cq in range(2):
                    rc = rcp.tile([64, 512], f32, tag="r", bufs=4,
                                  name=f"r{half}_{hh}_{cq}")
                    nc.vector.reciprocal(rc, ctx[cq][64:128, :])
                    nc.vector.tensor_mul(
                        ctxT_sb[mo:mo + 64, mt,
                                h0 + cq * 512:h0 + (cq + 1) * 512],
                        ctx[cq][0:64, :], rc)

            # ---- attention half 0, with Q-proj half 1 injected ----
            inject0 = bh1_quarter(2) + bh1_quarter(3)
            for hh in range(4):
                c_block(0, hh, inject0)
            for th in inject0:  # anything not yet dripped
                th()

            # ---- attention half 1, with out-proj half 0 injected ----
            inject1 = []
            for qi in range(8):
                inject1 += d_tile(qi, in_s_pool=False, tail=False)
            for hh in range(4):
                c_block(1, hh, inject1)
            for th in inject1:
                th()

            # ---- out-proj half 1 ----
            for qi in range(8, 16):
                for th in d_tile(qi, in_s_pool=True, tail=True):
                    th()

            rcp_cm.__exit__(None, None, None)
            otp_cm.__exit__(None, None, None)
            ptp_cm.__exit__(None, None, None)
            psC_cm.__exit__(None, None, None)
            psX_cm.__exit__(None, None, None)
            psS_cm.__exit__(None, None, None)
    nc.compile()
    return nc


def kernel(**inputs):
    global last_results, last_exec_wall_s
    from concourse.bass_utils import run_bass_kernel_spmd

    # BASS_TRACE needs the axon NTFF hook; disable tracing when the hook
    # module is unavailable so a stray env var cannot crash the run.
    if os.environ.get("BASS_TRACE"):
        try:
            from antenv import axon_hooks  # noqa: F401
        except Exception:
            os.environ["BASS_NEVER_TRACE"] = "1"

    q = np.asarray(inputs["queries"], dtype=np.float32)
    kx = np.asarray(inputs["keys"], dtype=np.float32)
    vx = np.asarray(inputs["values"], dtype=np.float32)
    vl = np.asarray(inputs["valid_lens"], dtype=np.int64).reshape(B)
    Wq = np.asarray(inputs["Wq"], dtype=np.float32)
    Wk = np.asarray(inputs["Wk"], dtype=np.float32)
    Wv = np.asarray(inputs["Wv"], dtype=np.float32)
    Wo = np.asarray(inputs["Wo"], dtype=np.float32)
    assert q.shape == (B, SQ, D) and kx.shape == (B, SK, D) and vx.shape == (B, SK, D)

    lens = np.clip(vl, 1, SK)
    lmax = int(lens.max())
    KT = (lmax + 127) // 128
    LK = KT * 128

    if KT not in _NC_CACHE:
        _NC_CACHE[KT] = _build(KT)
    nc = _NC_CACHE[KT]

    in_maps = []
    for c in range(N_CORES):
        b, hg = c // 4, c % 4
        cols = slice(DL * hg, DL * (hg + 1))
        m = np.where(np.arange(LK) < lens[b], 0.0, NEG).astype(np.float32)
        in_maps.append({
            "xqT": np.ascontiguousarray(q[b].T.astype(np.float16)),
            "xkT": np.ascontiguousarray(kx[b, :LK].T.astype(np.float16)),
            "xvT": np.ascontiguousarray(vx[b, :LK].T.astype(np.float16)),
            "wq": np.ascontiguousarray(Wq[:, cols].astype(np.float16)),
            "wk": np.ascontiguousarray(Wk[:, cols].astype(np.float16)),
            "wv": np.ascontiguousarray(Wv[:, cols].astype(np.float16)),
            "wo": np.ascontiguousarray(Wo[cols, :].astype(np.float16)),
            "mask": np.ascontiguousarray(m.reshape(KT, 128).T),
        })

    t0 = time.perf_counter()
    res = run_bass_kernel_spmd(nc, in_maps, core_ids=list(range(N_CORES)))
    last_exec_wall_s = time.perf_counter() - t0
    last_results = res

    outs = [res.results[c]["out"].astype(np.float32) for c in range(N_CORES)]
    full = np.stack([outs[0] + outs[1] + outs[2] + outs[3],
                     outs[4] + outs[5] + outs[6] + outs[7]])
    return full.astype(np.float32)


# revision 13
# speedup vs baseline: 1.1210x; 1.0445x over previous
"""Multi-head attention (B=2, S=2048, D=1024, H=16) on 8 Trainium2 cores.

Sharding: (batch, head-group-of-4) -> 8 cores, Megatron-style. Core c
handles batch c//4 and heads 4*(c%4)..4*(c%4)+3 (d_local = 256 columns of
Wq/Wk/Wv, 256 rows of Wo). Each core computes a partial [2048, 1024]
output; the host sums the 4 partials per batch (row-parallel Wo).

Key-side truncation: only ceil(max(valid_lens)/128) key tiles are ever
computed; invalid keys get a -1e6 bias on the ScalarE exp (scores are
computed transposed [key, query], denominator rides the ctx matmul as
64 "ones" columns of V'').

This build is organized as one deep pipeline so the ACT-engine exp
stream (the phase-C bottleneck) and the DMA streams hide under the
TensorE matmul stream (the global floor):
  warmup dummy matmuls (p-state ramp) -> K-proj (rides xk DMA) ->
  V-proj (rides per-key-tile xv DMA) -> Q-proj half 0 (rides xq) ->
  attention half 0 (with Q-proj half 1 matmuls injected into the
  per-key-tile slack) -> attention half 1 (with out-proj half 0
  injected) -> out-proj half 1, evacuations spread across ACT/DVE/Pool.

Precision: fp16 streams, fp16 single-pass matmuls, fp32 PSUM; partial
outputs return fp16 and are summed in fp32 on host.
"""
import sys
if "/opt/trn_rl_repo" not in sys.path:
    sys.path.insert(0, "/opt/trn_rl_repo")
import os
import time
import numpy as np

B, SQ, SK, D, H, HD = 2, 2048, 2048, 1024, 16, 64
NEG = -1.0e6
N_CORES = 8
DL = 256          # d_local: 4 heads * 64
KD = D // 128     # contraction tiles over D
N_WARM = int(os.environ.get("BASS_MHA_WARM", "6"))

_NC_CACHE = {}
last_results = None
last_exec_wall_s = None


def _chunks(total, cw):
    out = []
    c0 = 0
    while c0 < total:
        w = min(cw, total - c0)
        out.append((c0, w))
        c0 += w
    return out


def _build(KT):
    import concourse.bass as bass  # noqa: F401
    import concourse.tile as tile
    from concourse import bacc, mybir

    f32 = mybir.dt.float32
    f16 = mybir.dt.float16
    Exp = mybir.ActivationFunctionType.Exp
    LK = KT * 128

    nc = bacc.Bacc("TRN2", target_bir_lowering=False, debug=False,
                   num_devices=N_CORES)
    xqT = nc.dram_tensor("xqT", [D, SQ], f16, kind="ExternalInput")
    xkT = nc.dram_tensor("xkT", [D, LK], f16, kind="ExternalInput")
    xvT = nc.dram_tensor("xvT", [D, LK], f16, kind="ExternalInput")
    wq = nc.dram_tensor("wq", [D, DL], f16, kind="ExternalInput")
    wk = nc.dram_tensor("wk", [D, DL], f16, kind="ExternalInput")
    wv = nc.dram_tensor("wv", [D, DL], f16, kind="ExternalInput")
    wo = nc.dram_tensor("wo", [DL, D], f16, kind="ExternalInput")
    mask = nc.dram_tensor("mask", [128, KT], f32, kind="ExternalInput")
    out = nc.dram_tensor("out", [SQ, D], f16, kind="ExternalOutput")

    with tile.TileContext(nc) as tc:
        with tc.tile_pool(name="sg", bufs=1) as sg:
            wk_sb = sg.tile([128, KD, DL], f16)
            wq_sb = sg.tile([128, KD, DL], f16)
            wv_sb = sg.tile([128, KD, DL], f16)
            wo_sb = sg.tile([128, DL // 128, D], f16)
            mask_sb = sg.tile([128, KT], f32)
            kt_sb = sg.tile([128, 2, LK], f16)
            qt_sb = sg.tile([128, 2, SQ], f16)
            # V'' per head: [key, (v-tile | ones)] pairs per key tile
            v3_sb = sg.tile([128, 4, KT, 2, HD], f16)
            ctxT_sb = sg.tile([128, 2, SQ], f16)
            zero_sb = sg.tile([128, 512], f16)
            xk_sb = sg.tile([128, KD, LK], f16)
            xv_sb = sg.tile([128, KD, KT, 128], f16)
            xq_sb = sg.tile([128, KD, SQ], f16)
            warm_sb = sg.tile([1, 1], f32)

            # ---- DMA queue: arrival order is the pipeline order ----
            def dma_wk(c):
                nc.sync.dma_start(
                    out=wk_sb[:, c * 4:(c + 1) * 4, :],
                    in_=wk[c * 512:(c + 1) * 512, :].rearrange(
                        "(k p) j -> p k j", p=128))

            def dma_xk(c):
                nc.sync.dma_start(
                    out=xk_sb[:, 2 * c:2 * c + 2, :],
                    in_=xkT[c * 256:(c + 1) * 256, :].rearrange(
                        "(k p) j -> p k j", p=128))

            def dma_xq(half, c):
                nc.sync.dma_start(
                    out=xq_sb[:, 2 * c:2 * c + 2,
                              half * 1024:(half + 1) * 1024],
                    in_=xqT[c * 256:(c + 1) * 256,
                            half * 1024:(half + 1) * 1024].rearrange(
                                "(k p) j -> p k j", p=128))

            nc.sync.dma_start(out=mask_sb, in_=mask[:, :])
            dma_wk(0)
            dma_xk(0)
            dma_xk(1)
            dma_wk(1)
            dma_xk(2)
            dma_xk(3)
            nc.sync.dma_start(out=wq_sb,
                              in_=wq[:, :].rearrange("(k p) j -> p k j", p=128))
            for c in range(4):
                dma_xq(0, c)
            nc.sync.dma_start(out=wv_sb,
                              in_=wv[:, :].rearrange("(k p) j -> p k j", p=128))
            for t in range(KT):
                nc.sync.dma_start(
                    out=xv_sb[:, :, t, :],
                    in_=xvT[:, t * 128:(t + 1) * 128].rearrange(
                        "(k p) j -> p k j", p=128))
            for c in range(4):
                dma_xq(1, c)
            nc.sync.dma_start(out=wo_sb,
                              in_=wo[:, :].rearrange("(k p) j -> p k j", p=128))

            nc.gpsimd.memset(zero_sb, 0.0)
            nc.vector.memset(v3_sb, 1.0)  # ones slots; v slots overwritten
            # preload the exp activation table while DMAs stream
            nc.scalar.activation(warm_sb, mask_sb[0:1, 0:1], Exp)

            # ---- warmup: ramp the PE p-state during the DMA lead-in ----
            if N_WARM:
                wp_cm = tc.tile_pool(name="wp", bufs=1, space="PSUM")
                wp = wp_cm.__enter__()
                wps = wp.tile([128, 512], f32, tag="w")
                for _ in range(N_WARM):
                    nc.tensor.matmul(wps, zero_sb[:, 0:128], zero_sb,
                                     start=True, stop=True,
                                     skip_group_check=True)
                wp_cm.__exit__(None, None, None)

            # ---- A: K^T = (Wk^T blocks) @ Xk^T, k-major to ride xk DMA ----
            # psS is opened first so psA can close innermost (LIFO pools);
            # emission into psS starts only at phase B.
            psS_cm = tc.tile_pool(name="psS", bufs=1, space="PSUM")
            psS = psS_cm.__enter__()
            psA_cm = tc.tile_pool(name="psA", bufs=1, space="PSUM")
            psA = psA_cm.__enter__()
            for sec0, secw in _chunks(LK, 1024):
                am = [psA.tile([128, 1024], f32, tag="a", bufs=2,
                               name=f"a{m}_{sec0}") for m in range(2)]
                for k in range(KD):
                    for m in range(2):
                        for c0, cw in _chunks(secw, 512):
                            nc.tensor.matmul(
                                am[m][:, c0:c0 + cw],
                                wk_sb[:, k, m * 128:(m + 1) * 128],
                                xk_sb[:, k, sec0 + c0:sec0 + c0 + cw],
                                start=(k == 0), stop=(k == KD - 1))
                for m in range(2):
                    for c0, cw in _chunks(secw, 512):
                        nc.vector.tensor_copy(
                            kt_sb[:, m, sec0 + c0:sec0 + c0 + cw],
                            am[m][:, c0:c0 + cw])

            # ---- B half 0: Q^T cols 0:1024, k-major to ride xq DMA ----
            qp = [psS.tile([128, 1024], f32, tag="s", bufs=2, name=f"q{m}")
                  for m in range(2)]
            for k in range(KD):
                for m in range(2):
                    for cq in range(2):
                        nc.tensor.matmul(
                            qp[m][:, cq * 512:(cq + 1) * 512],
                            wq_sb[:, k, m * 128:(m + 1) * 128],
                            xq_sb[:, k, cq * 512:(cq + 1) * 512],
                            start=(k == 0), stop=(k == KD - 1))
            for m in range(2):
                nc.scalar.copy(qt_sb[:, m, 0:1024], qp[m])

            # ---- V-proj: per key tile, rides the per-tile xv DMA ----
            for tp in range(0, KT, 2):
                nj = min(2, KT - tp)
                vt = psA.tile([128, 2, 4, HD], f32, tag="a", bufs=2,
                              name=f"v{tp}")
                for j in range(nj):
                    t = tp + j
                    for k in range(KD):
                        nc.tensor.matmul(vt[:, j], xv_sb[:, k, t, :],
                                         wv_sb[:, k, :],
                                         start=(k == 0), stop=(k == KD - 1),
                                         skip_group_check=True)
                for j in range(nj):
                    t = tp + j
                    nc.vector.tensor_copy(v3_sb[:, :, t, 0, :], vt[:, j])
            psA_cm.__exit__(None, None, None)

            psX_cm = tc.tile_pool(name="psX", bufs=1, space="PSUM")
            psX = psX_cm.__enter__()
            psC_cm = tc.tile_pool(name="psC", bufs=1, space="PSUM")
            psC = psC_cm.__enter__()
            ptp_cm = tc.tile_pool(name="ptp", bufs=1)
            ptp = ptp_cm.__enter__()
            otp_cm = tc.tile_pool(name="otp", bufs=1)
            otp = otp_cm.__enter__()
            rcp_cm = tc.tile_pool(name="rcp", bufs=1)
            rcp = rcp_cm.__enter__()

            # Q-proj matmuls for half 1 (cols 1024:2048), injected one at a
            # time into attention-half-0 slack; rides the late xq DMA.
            def bh1_quarter(qq):
                xp = psX.tile([128, 2, 512], f32, tag="x", name=f"b1_{qq}")
                thunks = []
                for k in range(KD):
                    for m in range(2):
                        def mm(k=k, m=m):
                            nc.tensor.matmul(
                                xp[:, m, :],
                                wq_sb[:, k, m * 128:(m + 1) * 128],
                                xq_sb[:, k, qq * 512:(qq + 1) * 512],
                                start=(k == 0), stop=(k == KD - 1),
                                skip_group_check=True)
                        thunks.append(mm)

                def evac(qq=qq, xp=xp):
                    nc.vector.tensor_copy(qt_sb[:, :, qq * 512:(qq + 1) * 512],
                                          xp)
                thunks.append(evac)
                return thunks

            # out-proj for one query tile; evac split DVE/Pool (half 0,
            # injected under attention half 1) or ACT/DVE (half 1 tail).
            def d_tile(qi, in_s_pool, tail):
                if in_s_pool:
                    ov = psS.tile([128, 2, 512], f32, tag="s", bufs=2,
                                  name=f"o{qi}")
                else:
                    ov = psX.tile([128, 2, 512], f32, tag="x", name=f"o{qi}")
                thunks = []
                for kk in range(2):
                    for n in range(2):
                        def mm(n=n, kk=kk):
                            nc.tensor.matmul(
                                ov[:, n, :],
                                ctxT_sb[:, kk, qi * 128:(qi + 1) * 128],
                                wo_sb[:, kk, n * 512:(n + 1) * 512],
                                start=(kk == 0), stop=(kk == 1),
                                skip_group_check=True)
                        thunks.append(mm)

                def evac():
                    ot = otp.tile([128, 2, 512], f16, tag="ot", bufs=4,
                                  name=f"ot{qi}")
                    if tail:
                        nc.scalar.copy(ot[:, 0, :], ov[:, 0, :])
                        nc.vector.tensor_copy(ot[:, 1, :], ov[:, 1, :])
                    else:
                        nc.vector.tensor_copy(ot, ov)
                    nc.sync.dma_start(out=out[qi * 128:(qi + 1) * 128, :],
                                      in_=ot.rearrange("p a b -> p (a b)"))
                thunks.append(evac)
                return thunks

            # ---- attention block for (half, head): scores -> exp -> ctx ----
            def c_block(half, hh, inject):
                mt, mo = hh // 2, 64 * (hh % 2)
                h0 = half * 1024
                ctx = [psC.tile([128, 512], f32, tag="c", bufs=2,
                                name=f"c{half}_{hh}_{cq}") for cq in range(2)]
                sts, pts = [], []

                def s_step(t):
                    st = psS.tile([128, 1024], f32, tag="s", bufs=2,
                                  name=f"s{half}_{hh}_{t}")
                    for cq in range(2):
                        nc.tensor.matmul(
                            st[:, cq * 512:(cq + 1) * 512],
                            kt_sb[mo:mo + 64, mt, t * 128:(t + 1) * 128],
                            qt_sb[mo:mo + 64, mt,
                                  h0 + cq * 512:h0 + (cq + 1) * 512],
                            start=True, stop=True)
                    sts.append(st)

                def e_step(t):
                    pt = ptp.tile([128, 1024], f16, tag="pt", bufs=6,
                                  name=f"p{half}_{hh}_{t}")
                    nc.scalar.activation(pt, sts[t], Exp,
                                         bias=mask_sb[:, t:t + 1], scale=0.125)
                    pts.append(pt)

                def c_step(t):
                    for cq in range(2):
                        nc.tensor.matmul(
                            ctx[cq], v3_sb[:, hh, t, :, :],
                            pts[t][:, cq * 512:(cq + 1) * 512],
                            start=(t == 0), stop=(t == KT - 1),
                            skip_group_check=True)

                def drip(n):
                    for _ in range(n):
                        if inject:
                            inject.pop(0)()

                s_step(0)
                for t in range(1, KT):
                    s_step(t)
                    e_step(t - 1)
                    c_step(t - 1)
                    drip(2)
                e_step(KT - 1)
                c_step(KT - 1)
                drip(2)
                for cq in range(2):
                    rc = rcp.tile([64, 512], f32, tag="r", bufs=4,
                                  name=f"r{half}_{hh}_{cq}")
                    nc.vector.reciprocal(rc, ctx[cq][64:128, :])
                    nc.vector.tensor_mul(
                        ctxT_sb[mo:mo + 64, mt,
                                h0 + cq * 512:h0 + (cq + 1) * 512],
                        ctx[cq][0:64, :], rc)

            # ---- attention half 0, with Q-proj half 1 injected ----
            inject0 = bh1_quarter(2) + bh1_quarter(3)
            for hh in range(4):
                c_block(0, hh, inject0)
            for th in inject0:  # anything not yet dripped
                th()

            # ---- attention half 1, with out-proj half 0 injected ----
            inject1 = []
            for qi in range(8):
                inject1 += d_tile(qi, in_s_pool=False, tail=False)
            for hh in range(4):
                c_block(1, hh, inject1)
            for th in inject1:
                th()

            # ---- out-proj half 1 (alternate PSUM pools: 3-deep pipeline) ----
            for qi in range(8, 16):
                for th in d_tile(qi, in_s_pool=(qi % 2 == 0), tail=True):
                    th()

            rcp_cm.__exit__(None, None, None)
            otp_cm.__exit__(None, None, None)
            ptp_cm.__exit__(None, None, None)
            psC_cm.__exit__(None, None, None)
            psX_cm.__exit__(None, None, None)
            psS_cm.__exit__(None, None, None)
    nc.compile()
    return nc


def kernel(**inputs):
    global last_results, last_exec_wall_s
    from concourse.bass_utils import run_bass_kernel_spmd

    # BASS_TRACE needs the axon NTFF hook; disable tracing when the hook
    # module is unavailable so a stray env var cannot crash the run.
    if os.environ.get("BASS_TRACE"):
        try:
            from antenv import axon_hooks  # noqa: F401
        except Exception:
            os.environ["BASS_NEVER_TRACE"] = "1"

    q = np.asarray(inputs["queries"], dtype=np.float32)
    kx = np.asarray(inputs["keys"], dtype=np.float32)
    vx = np.asarray(inputs["values"], dtype=np.float32)
    vl = np.asarray(inputs["valid_lens"], dtype=np.int64).reshape(B)
    Wq = np.asarray(inputs["Wq"], dtype=np.float32)
    Wk = np.asarray(inputs["Wk"], dtype=np.float32)
    Wv = np.asarray(inputs["Wv"], dtype=np.float32)
    Wo = np.asarray(inputs["Wo"], dtype=np.float32)
    assert q.shape == (B, SQ, D) and kx.shape == (B, SK, D) and vx.shape == (B, SK, D)

    lens = np.clip(vl, 1, SK)
    lmax = int(lens.max())
    KT = (lmax + 127) // 128
    LK = KT * 128

    if KT not in _NC_CACHE:
        _NC_CACHE[KT] = _build(KT)
    nc = _NC_CACHE[KT]

    in_maps = []
    for c in range(N_CORES):
        b, hg = c // 4, c % 4
        cols = slice(DL * hg, DL * (hg + 1))
        m = np.where(np.arange(LK) < lens[b], 0.0, NEG).astype(np.float32)
        in_maps.append({
            "xqT": np.ascontiguousarray(q[b].T.astype(np.float16)),
            "xkT": np.ascontiguousarray(kx[b, :LK].T.astype(np.float16)),
            "xvT": np.ascontiguousarray(vx[b, :LK].T.astype(np.float16)),
            "wq": np.ascontiguousarray(Wq[:, cols].astype(np.float16)),
            "wk": np.ascontiguousarray(Wk[:, cols].astype(np.float16)),
            "wv": np.ascontiguousarray(Wv[:, cols].astype(np.float16)),
            "wo": np.ascontiguousarray(Wo[cols, :].astype(np.float16)),
            "mask": np.ascontiguousarray(m.reshape(KT, 128).T),
        })

    t0 = time.perf_counter()
    res = run_bass_kernel_spmd(nc, in_maps, core_ids=list(range(N_CORES)))
    last_exec_wall_s = time.perf_counter() - t0
    last_results = res

    outs = [res.results[c]["out"].astype(np.float32) for c in range(N_CORES)]
    full = np.stack([outs[0] + outs[1] + outs[2] + outs[3],
                     outs[4] + outs[5] + outs[6] + outs[7]])
    return full.astype(np.float32)


# revision 18
# speedup vs baseline: 1.1612x; 1.0358x over previous
"""Multi-head attention (B=2, S=2048, D=1024, H=16) on 8 Trainium2 cores.

Sharding: (batch, head-group-of-4) -> 8 cores, Megatron-style. Core c
handles batch c//4 and heads 4*(c%4)..4*(c%4)+3 (d_local = 256 columns of
Wq/Wk/Wv, 256 rows of Wo). Each core computes a partial [2048, 1024]
output; the host sums the 4 partials per batch (row-parallel Wo).

Key-side truncation: only ceil(max(valid_lens)/128) key tiles are ever
computed; invalid keys get a -1e6 bias on the ScalarE exp (scores are
computed transposed [key, query], denominator rides the ctx matmul as
64 "ones" columns of V'').

This build is organized as one deep pipeline so the ACT-engine exp
stream (the phase-C bottleneck) and the DMA streams hide under the
TensorE matmul stream (the global floor):
  warmup dummy matmuls (p-state ramp) -> K-proj (rides xk DMA) ->
  V-proj (rides per-key-tile xv DMA) -> Q-proj half 0 (rides xq) ->
  attention half 0 (with Q-proj half 1 matmuls injected into the
  per-key-tile slack) -> attention half 1 (with out-proj half 0
  injected) -> out-proj half 1, evacuations spread across ACT/DVE/Pool.

Precision: fp16 streams, fp16 single-pass matmuls, fp32 PSUM; partial
outputs return fp16 and are summed in fp32 on host.
"""
import sys
if "/opt/trn_rl_repo" not in sys.path:
    sys.path.insert(0, "/opt/trn_rl_repo")
import os
import time
import numpy as np

B, SQ, SK, D, H, HD = 2, 2048, 2048, 1024, 16, 64
NEG = -1.0e6
N_CORES = 8
DL = 256          # d_local: 4 heads * 64
KD = D // 128     # contraction tiles over D
N_WARM = int(os.environ.get("BASS_MHA_WARM", "5"))

_NC_CACHE = {}
last_results = None
last_exec_wall_s = None


def _chunks(total, cw):
    out = []
    c0 = 0
    while c0 < total:
        w = min(cw, total - c0)
        out.append((c0, w))
        c0 += w
    return out


def _build(KT):
    import concourse.bass as bass  # noqa: F401
    import concourse.tile as tile
    from concourse import bacc, mybir

    f32 = mybir.dt.float32
    f16 = mybir.dt.float16
    Exp = mybir.ActivationFunctionType.Exp
    LK = KT * 128

    nc = bacc.Bacc("TRN2", target_bir_lowering=False, debug=False,
                   num_devices=N_CORES)
    xqT = nc.dram_tensor("xqT", [D, SQ], f16, kind="ExternalInput")
    xkT = nc.dram_tensor("xkT", [D, LK], f16, kind="ExternalInput")
    xvT = nc.dram_tensor("xvT", [D, LK], f16, kind="ExternalInput")
    wq = nc.dram_tensor("wq", [D, DL], f16, kind="ExternalInput")
    wk = nc.dram_tensor("wk", [D, DL], f16, kind="ExternalInput")
    wv = nc.dram_tensor("wv", [D, DL], f16, kind="ExternalInput")
    wo = nc.dram_tensor("wo", [DL, D], f16, kind="ExternalInput")
    mask = nc.dram_tensor("mask", [128, KT], f32, kind="ExternalInput")
    out = nc.dram_tensor("out", [SQ, D], f16, kind="ExternalOutput")

    with tile.TileContext(nc) as tc:
        with tc.tile_pool(name="sg", bufs=1) as sg:
            wk_sb = sg.tile([128, KD, DL], f16)
            wq_sb = sg.tile([128, KD, DL], f16)
            wv_sb = sg.tile([128, KD, DL], f16)
            wo_sb = sg.tile([128, DL // 128, D], f16)
            mask_sb = sg.tile([128, KT], f32)
            kt_sb = sg.tile([128, 2, LK], f16)
            qt_sb = sg.tile([128, 2, SQ], f16)
            # V'' per head: [key, (v-tile | ones)] pairs per key tile
            v3_sb = sg.tile([128, 4, KT, 2, HD], f16)
            ctxT_sb = sg.tile([128, 2, SQ], f16)
            zero_sb = sg.tile([128, 512], f16)
            xk_sb = sg.tile([128, KD, LK], f16)
            xv_sb = sg.tile([128, KD, KT, 128], f16)
            xq_sb = sg.tile([128, KD, SQ], f16)
            warm_sb = sg.tile([1, 1], f32)

            # ---- DMA queue: arrival order is the pipeline order ----
            def dma_wk(c):
                nc.sync.dma_start(
                    out=wk_sb[:, c * 4:(c + 1) * 4, :],
                    in_=wk[c * 512:(c + 1) * 512, :].rearrange(
                        "(k p) j -> p k j", p=128))

            def dma_xk(c):
                nc.sync.dma_start(
                    out=xk_sb[:, 2 * c:2 * c + 2, :],
                    in_=xkT[c * 256:(c + 1) * 256, :].rearrange(
                        "(k p) j -> p k j", p=128))

            def dma_xq(half, c):
                nc.sync.dma_start(
                    out=xq_sb[:, 2 * c:2 * c + 2,
                              half * 1024:(half + 1) * 1024],
                    in_=xqT[c * 256:(c + 1) * 256,
                            half * 1024:(half + 1) * 1024].rearrange(
                                "(k p) j -> p k j", p=128))

            nc.sync.dma_start(out=mask_sb, in_=mask[:, :])
            dma_wk(0)
            dma_xk(0)
            dma_xk(1)
            dma_wk(1)
            nc.sync.dma_start(out=wq_sb,
                              in_=wq[:, :].rearrange("(k p) j -> p k j", p=128))
            dma_xk(2)
            dma_xk(3)
            for c in range(4):
                dma_xq(0, c)
            nc.sync.dma_start(out=wv_sb,
                              in_=wv[:, :].rearrange("(k p) j -> p k j", p=128))
            for t in range(KT):
                nc.sync.dma_start(
                    out=xv_sb[:, :, t, :],
                    in_=xvT[:, t * 128:(t + 1) * 128].rearrange(
                        "(k p) j -> p k j", p=128))
            for c in range(4):
                dma_xq(1, c)
            nc.sync.dma_start(out=wo_sb,
                              in_=wo[:, :].rearrange("(k p) j -> p k j", p=128))

            nc.gpsimd.memset(zero_sb, 0.0)
            nc.vector.memset(v3_sb, 1.0)  # ones slots; v slots overwritten
            # preload the exp activation table while DMAs stream
            nc.scalar.activation(warm_sb, mask_sb[0:1, 0:1], Exp)

            # ---- warmup: ramp the PE p-state during the DMA lead-in ----
            if N_WARM:
                wp_cm = tc.tile_pool(name="wp", bufs=1, space="PSUM")
                wp = wp_cm.__enter__()
                wps = wp.tile([128, 448], f32, tag="w")
                for _ in range(N_WARM):
                    nc.tensor.matmul(wps, zero_sb[:, 0:128],
                                     zero_sb[:, 0:448],
                                     start=True, stop=True,
                                     skip_group_check=True)
                wp_cm.__exit__(None, None, None)

            # ---- A: K^T = (Wk^T blocks) @ Xk^T, k-major to ride xk DMA ----
            # psS is opened first so psA can close innermost (LIFO pools);
            # emission into psS starts only at phase B.
            psS_cm = tc.tile_pool(name="psS", bufs=1, space="PSUM")
            psS = psS_cm.__enter__()
            psA_cm = tc.tile_pool(name="psA", bufs=1, space="PSUM")
            psA = psA_cm.__enter__()
            for sec0, secw in _chunks(LK, 1024):
                am = [psA.tile([128, 1024], f32, tag="a", bufs=2,
                               name=f"a{m}_{sec0}") for m in range(2)]
                for k in range(KD):
                    for m in range(2):
                        for c0, cw in _chunks(secw, 512):
                            nc.tensor.matmul(
                                am[m][:, c0:c0 + cw],
                                wk_sb[:, k, m * 128:(m + 1) * 128],
                                xk_sb[:, k, sec0 + c0:sec0 + c0 + cw],
                                start=(k == 0), stop=(k == KD - 1))
                for m in range(2):
                    for c0, cw in _chunks(secw, 512):
                        nc.vector.tensor_copy(
                            kt_sb[:, m, sec0 + c0:sec0 + c0 + cw],
                            am[m][:, c0:c0 + cw])

            # ---- B half 0: Q^T cols 0:1024, k-major to ride xq DMA ----
            qp = [psS.tile([128, 1024], f32, tag="s", bufs=2, name=f"q{m}")
                  for m in range(2)]
            for k in range(KD):
                for m in range(2):
                    for cq in range(2):
                        nc.tensor.matmul(
                            qp[m][:, cq * 512:(cq + 1) * 512],
                            wq_sb[:, k, m * 128:(m + 1) * 128],
                            xq_sb[:, k, cq * 512:(cq + 1) * 512],
                            start=(k == 0), stop=(k == KD - 1))
            for m in range(2):
                nc.scalar.copy(qt_sb[:, m, 0:1024], qp[m])

            # ---- V-proj: per key tile, rides the per-tile xv DMA ----
            for tp in range(0, KT, 2):
                nj = min(2, KT - tp)
                vt = psA.tile([128, 2, 4, HD], f32, tag="a", bufs=2,
                              name=f"v{tp}")
                for j in range(nj):
                    t = tp + j
                    for k in range(KD):
                        nc.tensor.matmul(vt[:, j], xv_sb[:, k, t, :],
                                         wv_sb[:, k, :],
                                         start=(k == 0), stop=(k == KD - 1),
                                         skip_group_check=True)
                for j in range(nj):
                    t = tp + j
                    nc.vector.tensor_copy(v3_sb[:, :, t, 0, :], vt[:, j])
            psA_cm.__exit__(None, None, None)

            psX_cm = tc.tile_pool(name="psX", bufs=1, space="PSUM")
            psX = psX_cm.__enter__()
            psC_cm = tc.tile_pool(name="psC", bufs=1, space="PSUM")
            psC = psC_cm.__enter__()
            ptp_cm = tc.tile_pool(name="ptp", bufs=1)
            ptp = ptp_cm.__enter__()
            otp_cm = tc.tile_pool(name="otp", bufs=1)
            otp = otp_cm.__enter__()
            rcp_cm = tc.tile_pool(name="rcp", bufs=1)
            rcp = rcp_cm.__enter__()

            # Q-proj matmuls for half 1 (cols 1024:2048), injected one at a
            # time into attention-half-0 slack; rides the late xq DMA.
            def bh1_quarter(qq):
                xp = psX.tile([128, 2, 512], f32, tag="x", name=f"b1_{qq}")
                thunks = []
                for k in range(KD):
                    for m in range(2):
                        def mm(k=k, m=m):
                            nc.tensor.matmul(
                                xp[:, m, :],
                                wq_sb[:, k, m * 128:(m + 1) * 128],
                                xq_sb[:, k, qq * 512:(qq + 1) * 512],
                                start=(k == 0), stop=(k == KD - 1),
                                skip_group_check=True)
                        thunks.append(mm)

                def evac(qq=qq, xp=xp):
                    nc.vector.tensor_copy(qt_sb[:, :, qq * 512:(qq + 1) * 512],
                                          xp)
                thunks.append(evac)
                return thunks

            # out-proj for one query tile; evac split DVE/Pool (half 0,
            # injected under attention half 1) or ACT/DVE (half 1 tail).
            def d_tile(qi, in_s_pool, tail):
                if in_s_pool:
                    ov = psS.tile([128, 2, 512], f32, tag="s", bufs=2,
                                  name=f"o{qi}")
                else:
                    ov = psX.tile([128, 2, 512], f32, tag="x", name=f"o{qi}")
                thunks = []
                for kk in range(2):
                    for n in range(2):
                        def mm(n=n, kk=kk):
                            nc.tensor.matmul(
                                ov[:, n, :],
                                ctxT_sb[:, kk, qi * 128:(qi + 1) * 128],
                                wo_sb[:, kk, n * 512:(n + 1) * 512],
                                start=(kk == 0), stop=(kk == 1),
                                skip_group_check=True)
                        thunks.append(mm)

                def evac():
                    ot = otp.tile([128, 2, 512], f16, tag="ot", bufs=4,
                                  name=f"ot{qi}")
                    if qi == 15:
                        # pipeline the final tile's evac halves with its DMA
                        nc.scalar.copy(ot[:, 0, :], ov[:, 0, :])
                        nc.sync.dma_start(
                            out=out[qi * 128:(qi + 1) * 128, 0:512],
                            in_=ot[:, 0, :])
                        nc.vector.tensor_copy(ot[:, 1, :], ov[:, 1, :])
                        nc.sync.dma_start(
                            out=out[qi * 128:(qi + 1) * 128, 512:1024],
                            in_=ot[:, 1, :])
                        return
                    if tail:
                        nc.scalar.copy(ot[:, 0, :], ov[:, 0, :])
                        nc.vector.tensor_copy(ot[:, 1, :], ov[:, 1, :])
                    elif qi % 2 == 0:
                        nc.scalar.copy(ot, ov)
                    else:
                        nc.vector.tensor_copy(ot, ov)
                    nc.sync.dma_start(out=out[qi * 128:(qi + 1) * 128, :],
                                      in_=ot.rearrange("p a b -> p (a b)"))
                thunks.append(evac)
                return thunks

            # ---- attention block for (half, head): scores -> exp -> ctx ----
            def c_block(half, hh, inject):
                mt, mo = hh // 2, 64 * (hh % 2)
                h0 = half * 1024
                ctx = [psC.tile([128, 512], f32, tag="c", bufs=2,
                                name=f"c{half}_{hh}_{cq}") for cq in range(2)]
                sts, pts = [], []

                def s_step(t):
                    st = psS.tile([128, 1024], f32, tag="s", bufs=2,
                                  name=f"s{half}_{hh}_{t}")
                    for cq in range(2):
                        nc.tensor.matmul(
                            st[:, cq * 512:(cq + 1) * 512],
                            kt_sb[mo:mo + 64, mt, t * 128:(t + 1) * 128],
                            qt_sb[mo:mo + 64, mt,
                                  h0 + cq * 512:h0 + (cq + 1) * 512],
                            start=True, stop=True)
                    sts.append(st)

                def e_step(t):
                    pt = ptp.tile([128, 1024], f16, tag="pt", bufs=6,
                                  name=f"p{half}_{hh}_{t}")
                    nc.scalar.activation(pt, sts[t], Exp,
                                         bias=mask_sb[:, t:t + 1], scale=0.125)
                    pts.append(pt)

                def c_step(t):
                    for cq in range(2):
                        nc.tensor.matmul(
                            ctx[cq], v3_sb[:, hh, t, :, :],
                            pts[t][:, cq * 512:(cq + 1) * 512],
                            start=(t == 0), stop=(t == KT - 1),
                            skip_group_check=True)

                def drip(n):
                    for _ in range(n):
                        if inject:
                            inject.pop(0)()

                s_step(0)
                for t in range(1, KT):
                    s_step(t)
                    e_step(t - 1)
                    c_step(t - 1)
                    drip(2)
                e_step(KT - 1)
                c_step(KT - 1)
                drip(2)
                for cq in range(2):
                    rc = rcp.tile([64, 512], f32, tag="r", bufs=4,
                                  name=f"r{half}_{hh}_{cq}")
                    nc.vector.reciprocal(rc, ctx[cq][64:128, :])
                    nc.vector.tensor_mul(
                        ctxT_sb[mo:mo + 64, mt,
                                h0 + cq * 512:h0 + (cq + 1) * 512],
                        ctx[cq][0:64, :], rc)

            # ---- attention half 0, with Q-proj half 1 injected ----
            inject0 = bh1_quarter(2) + bh1_quarter(3)
            for hh in range(4):
                c_block(0, hh, inject0)
            for th in inject0:  # anything not yet dripped
                th()

            # ---- attention half 1, with out-proj half 0 injected ----
            inject1 = []
            for qi in range(8):
                inject1 += d_tile(qi, in_s_pool=False, tail=False)
            for hh in range(4):
                c_block(1, hh, inject1)
            for th in inject1:
                th()

            # ---- out-proj half 1 (s,s,x PSUM pattern: 3-deep pipeline) ----
            for qi in range(8, 16):
                for th in d_tile(qi, in_s_pool=((qi - 8) % 3 != 2), tail=True):
                    th()

            rcp_cm.__exit__(None, None, None)
            otp_cm.__exit__(None, None, None)
            ptp_cm.__exit__(None, None, None)
            psC_cm.__exit__(None, None, None)
            psX_cm.__exit__(None, None, None)
            psS_cm.__exit__(None, None, None)
    nc.compile()
    return nc


def kernel(**inputs):
    global last_results, last_exec_wall_s
    from concourse.bass_utils import run_bass_kernel_spmd

    # BASS_TRACE needs the axon NTFF hook; disable tracing when the hook
    # module is unavailable so a stray env var cannot crash the run.
    if os.environ.get("BASS_TRACE"):
        try:
            from antenv import axon_hooks  # noqa: F401
        except Exception:
            os.environ["BASS_NEVER_TRACE"] = "1"

    q = np.asarray(inputs["queries"], dtype=np.float32)
    kx = np.asarray(inputs["keys"], dtype=np.float32)
    vx = np.asarray(inputs["values"], dtype=np.float32)
    vl = np.asarray(inputs["valid_lens"], dtype=np.int64).reshape(B)
    Wq = np.asarray(inputs["Wq"], dtype=np.float32)
    Wk = np.asarray(inputs["Wk"], dtype=np.float32)
    Wv = np.asarray(inputs["Wv"], dtype=np.float32)
    Wo = np.asarray(inputs["Wo"], dtype=np.float32)
    assert q.shape == (B, SQ, D) and kx.shape == (B, SK, D) and vx.shape == (B, SK, D)

    lens = np.clip(vl, 1, SK)
    lmax = int(lens.max())
    KT = (lmax + 127) // 128
    LK = KT * 128

    if KT not in _NC_CACHE:
        _NC_CACHE[KT] = _build(KT)
    nc = _NC_CACHE[KT]

    in_maps = []
    for c in range(N_CORES):
        b, hg = c // 4, c % 4
        cols = slice(DL * hg, DL * (hg + 1))
        m = np.where(np.arange(LK) < lens[b], 0.0, NEG).astype(np.float32)
        in_maps.append({
            "xqT": np.ascontiguousarray(q[b].T.astype(np.float16)),
            "xkT": np.ascontiguousarray(kx[b, :LK].T.astype(np.float16)),
            "xvT": np.ascontiguousarray(vx[b, :LK].T.astype(np.float16)),
            "wq": np.ascontiguousarray(Wq[:, cols].astype(np.float16)),
            "wk": np.ascontiguousarray(Wk[:, cols].astype(np.float16)),
            "wv": np.ascontiguousarray(Wv[:, cols].astype(np.float16)),
            "wo": np.ascontiguousarray(Wo[cols, :].astype(np.float16)),
            "mask": np.ascontiguousarray(m.reshape(KT, 128).T),
        })

    t0 = time.perf_counter()
    res = run_bass_kernel_spmd(nc, in_maps, core_ids=list(range(N_CORES)))
    last_exec_wall_s = time.perf_counter() - t0
    last_results = res

    outs = [res.results[c]["out"].astype(np.float32) for c in range(N_CORES)]
    full = np.stack([outs[0] + outs[1] + outs[2] + outs[3],
                     outs[4] + outs[5] + outs[6] + outs[7]])
    return full.astype(np.float32)
